# revision 3
# baseline (speedup 1.0000x reference)
# Multi-head causal attention (B=4, S=2048, D=1024, H=16, Dh=64) on 8 trn2 cores.
#
# Sharding: tensor-parallel over heads — core c owns heads (2c, 2c+1) for all
# batches. Each core projects Q/K/V for its 2 heads, runs causal attention, and
# computes a partial output projection against its 128 rows of w_o. The host
# sums the 8 partial outputs (the "all-reduce").
#
# Layouts (chosen so no on-device transposes are needed):
#   embedT  [B, 128, 8, S]  bf16   embedT[b, p, dc, s] = embed[b, s, dc*128+p]
#   wq2/wk2/wv2 [128, 8, 128] bf16 (per-core 2-head slice; wq pre-scaled 1/8)
#   wo2     [128, 1024] bf16       (per-core 128 rows of w_o)
#   Scores are computed transposed: sT[k, q] = sum_dh kT[dh,k] qT[dh,q], so the
#   softmax denominator comes from a ones-column appended to V (PV matmul
#   accumulates both the PV product and the exp-sum), and exp'd scores feed the
#   PV matmul directly as the moving operand — no transposes anywhere.
import numpy as np
import ml_dtypes

B, S, D, H, Dh = 4, 2048, 1024, 16, 64
NCORES = 8
HPC = H // NCORES          # heads per core = 2
DC = D // 128              # d chunks = 8
NQB = S // 512             # q blocks = 4
NKB = S // 128             # k chunks = 16
NST = S // 128             # s tiles = 16
NEG = -1.0e30

_cache = {}


def _build_nc():
    import concourse.bass as bass
    import concourse.mybir as mybir
    import concourse.tile as tile
    from concourse import bacc

    bf16 = mybir.dt.bfloat16
    f32 = mybir.dt.float32
    EXP = mybir.ActivationFunctionType.Exp

    nc = bacc.Bacc("TRN2", target_bir_lowering=False, debug=False,
                   num_devices=NCORES)

    embedT = nc.dram_tensor("embedT", [B, 128, DC, S], bf16, kind="ExternalInput")
    wq2 = nc.dram_tensor("wq2", [128, DC, 128], bf16, kind="ExternalInput")
    wk2 = nc.dram_tensor("wk2", [128, DC, 128], bf16, kind="ExternalInput")
    wv2 = nc.dram_tensor("wv2", [128, DC, 128], bf16, kind="ExternalInput")
    wo2 = nc.dram_tensor("wo2", [128, D], bf16, kind="ExternalInput")
    maskin = nc.dram_tensor("maskin", [128, 512], f32, kind="ExternalInput")
    outp = nc.dram_tensor("outp", [B, S, D], f32, kind="ExternalOutput")

    with tile.TileContext(nc) as tc:
        with (
            tc.tile_pool(name="const", bufs=1) as const,
            tc.tile_pool(name="etp", bufs=2) as etp,
            tc.tile_pool(name="qkp", bufs=2) as qkp,
            tc.tile_pool(name="vxp", bufs=2) as vxp,
            tc.tile_pool(name="hdp", bufs=2) as hdp,
            tc.tile_pool(name="expp", bufs=3) as expp,
            tc.tile_pool(name="denp", bufs=3) as denp,
            tc.tile_pool(name="outs", bufs=3) as outs,
            tc.tile_pool(name="pscore", bufs=2, space="PSUM") as pscore,
            tc.tile_pool(name="ppv", bufs=2, space="PSUM") as ppv,
            tc.tile_pool(name="drp", bufs=3, space="DRAM") as drp,
        ):
            mask_sb = const.tile([128, 512], f32, tag="mask")
            nc.sync.dma_start(out=mask_sb[:], in_=maskin[:])
            wq_sb = const.tile([128, DC, 128], bf16, tag="wq")
            wk_sb = const.tile([128, DC, 128], bf16, tag="wk")
            wv_sb = const.tile([128, DC, 128], bf16, tag="wv")
            wo_sb = const.tile([128, D], bf16, tag="wo")
            nc.sync.dma_start(out=wq_sb[:], in_=wq2[:])
            nc.sync.dma_start(out=wk_sb[:], in_=wk2[:])
            nc.sync.dma_start(out=wv_sb[:], in_=wv2[:])
            nc.sync.dma_start(out=wo_sb[:], in_=wo2[:])

            for b in range(B):
                et = etp.tile([128, DC, S], bf16, tag="et")
                nc.sync.dma_start(out=et[:], in_=embedT[b])

                # ---- Q/K projections -> qT2/kT2 [128(2h x 64dh), S] bf16
                qT2 = qkp.tile([128, S], bf16, tag="qT2")
                kT2 = qkp.tile([128, S], bf16, tag="kT2")
                for dst, w_sb in ((qT2, wq_sb), (kT2, wk_sb)):
                    for sblk in range(S // 512):
                        ps = pscore.tile([128, 1024], f32, tag="score")
                        for dc in range(DC):
                            nc.tensor.matmul(
                                ps[:, 0:512], w_sb[:, dc, :],
                                et[:, dc, sblk * 512:(sblk + 1) * 512],
                                start=(dc == 0), stop=(dc == DC - 1),
                            )
                        nc.scalar.copy(
                            out=dst[:, sblk * 512:(sblk + 1) * 512],
                            in_=ps[:, 0:512])

                # ---- V projection -> natural layout + ones col
                # vext[h] [128(k), NKB, 65] bf16
                vext0 = vxp.tile([128, NKB, 65], bf16, tag="vext0")
                vext1 = vxp.tile([128, NKB, 65], bf16, tag="vext1")
                nc.vector.memset(vext0[:, :, 64:65], 1.0)
                nc.vector.memset(vext1[:, :, 64:65], 1.0)
                for st in range(NST):
                    ps = pscore.tile([128, 1024], f32, tag="score")
                    for dc in range(DC):
                        nc.tensor.matmul(
                            ps[:, 0:128],
                            et[:, dc, st * 128:(st + 1) * 128],
                            wv_sb[:, dc, :],
                            start=(dc == 0), stop=(dc == DC - 1),
                        )
                    nc.vector.tensor_copy(out=vext0[:, st, 0:64], in_=ps[:, 0:64])
                    nc.vector.tensor_copy(out=vext1[:, st, 0:64], in_=ps[:, 64:128])

                # ---- attention, 2 heads, causal, flash-style over k chunks
                headT2 = hdp.tile([128, S], bf16, tag="headT2")
                for qb in range(NQB):
                    qs = slice(qb * 512, (qb + 1) * 512)
                    pv0 = ppv.tile([65, 512], f32, tag="pv0")
                    pv1 = ppv.tile([65, 512], f32, tag="pv1")
                    nkb = 4 * qb + 4
                    for kb in range(nkb):
                        ps = pscore.tile([128, 1024], f32, tag="score")
                        ks = slice(kb * 128, (kb + 1) * 128)
                        # scoresT[k, q] per head; heads row-packed on PE
                        nc.tensor.matmul(ps[:, 0:512], kT2[0:64, ks],
                                         qT2[0:64, qs])
                        nc.tensor.matmul(ps[:, 512:1024], kT2[64:128, ks],
                                         qT2[64:128, qs])
                        if kb >= 4 * qb:
                            r = kb - 4 * qb
                            w = (r + 1) * 128
                            moff = 384 - r * 128
                            for h in (0, 1):
                                nc.vector.tensor_add(
                                    ps[:, h * 512:h * 512 + w],
                                    ps[:, h * 512:h * 512 + w],
                                    mask_sb[:, moff:moff + w],
                                )
                        ex = expp.tile([128, 1024], bf16, tag="ex")
                        nc.scalar.activation(out=ex[:], in_=ps[:], func=EXP)
                        first, last = (kb == 0), (kb == nkb - 1)
                        nc.tensor.matmul(pv0[:], vext0[:, kb, :],
                                         ex[:, 0:512], start=first, stop=last)
                        nc.tensor.matmul(pv1[:], vext1[:, kb, :],
                                         ex[:, 512:1024], start=first, stop=last)
                    # normalize: headT2[h*64:(h+1)*64, qs] = pv[0:64] / den
                    for h, pv in ((0, pv0), (1, pv1)):
                        den = denp.tile([128, 512], f32, tag="den")
                        nc.vector.reciprocal(out=den[64:65, :], in_=pv[64:65, :])
                        dden = drp.tile([1, 512], f32, tag="dden")
                        nc.sync.dma_start(out=dden[:], in_=den[64:65, :])
                        bcap = bass.AP(tensor=dden.tensor, offset=dden.offset,
                                       ap=[[0, 64], [1, 512]])
                        nc.sync.dma_start(out=den[0:64, :], in_=bcap)
                        nc.vector.tensor_mul(
                            headT2[h * 64:(h + 1) * 64, qs],
                            pv[0:64, :], den[0:64, :])

                # ---- output projection (partial): out[b] += headT2.T @ wo2
                for st in range(NST):
                    po = pscore.tile([128, 1024], f32, tag="score")
                    hs = headT2[:, st * 128:(st + 1) * 128]
                    nc.tensor.matmul(po[:, 0:512], hs, wo_sb[:, 0:512])
                    nc.tensor.matmul(po[:, 512:1024], hs, wo_sb[:, 512:1024])
                    ob = outs.tile([128, 1024], f32, tag="ob")
                    if st % 2 == 0:
                        nc.vector.tensor_copy(out=ob[:], in_=po[:])
                    else:
                        nc.scalar.copy(out=ob[:], in_=po[:])
                    nc.sync.dma_start(
                        out=outp[b, st * 128:(st + 1) * 128, :], in_=ob[:])

    nc.compile()
    return nc


def _host_prep(embed, w_q, w_k, w_v, w_o):
    bf = ml_dtypes.bfloat16
    embedT = np.ascontiguousarray(
        embed.reshape(B, S, DC, 128).transpose(0, 3, 2, 1)).astype(bf)
    # mask: bigM[k, j] = NEG if j < 384 + k else 0
    j = np.arange(512)[None, :]
    k = np.arange(128)[:, None]
    mask = np.where(j < 384 + k, np.float32(NEG), np.float32(0.0))
    mask = np.ascontiguousarray(mask.astype(np.float32))

    in_maps = []
    for c in range(NCORES):
        h0, h1 = HPC * c, HPC * c + 1
        wq_cat = np.concatenate([w_q[h0], w_q[h1]], axis=1) * (1.0 / 8.0)
        wk_cat = np.concatenate([w_k[h0], w_k[h1]], axis=1)
        wv_cat = np.concatenate([w_v[h0], w_v[h1]], axis=1)
        def lay(w):  # [1024, 128] -> [128, DC, 128]
            return np.ascontiguousarray(
                w.reshape(DC, 128, 128).transpose(1, 0, 2)).astype(bf)
        in_maps.append({
            "embedT": embedT,
            "wq2": lay(wq_cat),
            "wk2": lay(wk_cat),
            "wv2": lay(wv_cat),
            "wo2": np.ascontiguousarray(
                w_o[128 * c:128 * (c + 1), :]).astype(bf),
            "maskin": mask,
        })
    return in_maps


def kernel(embed, pad_mask, w_q, w_k, w_v, w_o, _trace=False):
    from concourse.bass_utils import run_bass_kernel_spmd

    embed = np.asarray(embed, dtype=np.float32)
    w_q = np.asarray(w_q, dtype=np.float32)
    w_k = np.asarray(w_k, dtype=np.float32)
    w_v = np.asarray(w_v, dtype=np.float32)
    w_o = np.asarray(w_o, dtype=np.float32)

    if "nc" not in _cache:
        _cache["nc"] = _build_nc()
    nc = _cache["nc"]

    in_maps = _host_prep(embed, w_q, w_k, w_v, w_o)
    res = run_bass_kernel_spmd(nc, in_maps, core_ids=list(range(NCORES)),
                               trace=_trace)
    _cache["last_result"] = res
    out = np.zeros((B, S, D), dtype=np.float32)
    for r in res.results:
        out += r["outp"]
    return out
